# revision 1
# baseline (speedup 1.0000x reference)
"""DeepseekV2 MoE (T=512, H=2048, I=1408, E=16, top-6 group-limited routing)
on 8 trn2 NeuronCores, expert-parallel (2 experts/core) with on-device fp32
routing, bf16 expert GEMMs, and a ReduceScatter combine.

Self-contained: hardcodes all shapes; builds one SPMD Bass program shared by
all 8 cores (per-core inputs carry that core's expert weight slices and a
one-hot selector mapping its experts to router columns).
"""

import numpy as np

import concourse.bass as bass
import concourse.mybir as mybir
import concourse.tile as tile
from concourse import bacc
from concourse.bass_utils import run_bass_kernel_spmd
from concourse.masks import make_identity

F32 = mybir.dt.float32
BF16 = mybir.dt.bfloat16
AF = mybir.ActivationFunctionType
OP = mybir.AluOpType

T, H, I, E = 512, 2048, 1408, 16
P = 128
NCORES = 8
EPC = E // NCORES          # experts per core = 2
NKT = H // P               # 16 k-tiles over H
NIB = I // P               # 11 k-tiles over I
NTT = T // P               # 4 token tiles
RSF = 2.5
BIG = 1.0e30


def _bcast_ap(ap, parts=P):
    """Partition-broadcast a 1D AP to [parts, n]."""
    return bass.AP(tensor=ap.tensor, offset=ap.offset, ap=[[0, parts]] + list(ap.ap))


def build_nc(use_rs=True, stages=4, ncores=NCORES):
    nc = bacc.Bacc("TRN2", target_bir_lowering=False, debug=False,
                   num_devices=ncores)

    x_d = nc.dram_tensor("x", [T, H], F32, kind="ExternalInput")
    gw_d = nc.dram_tensor("gw", [E, H], F32, kind="ExternalInput")
    cb_d = nc.dram_tensor("cb", [E], F32, kind="ExternalInput")
    wg_d = nc.dram_tensor("wg", [EPC, I, H], F32, kind="ExternalInput")
    wu_d = nc.dram_tensor("wu", [EPC, I, H], F32, kind="ExternalInput")
    wd_d = nc.dram_tensor("wd", [EPC, H, I], F32, kind="ExternalInput")
    esel_d = nc.dram_tensor("esel", [EPC, E], F32, kind="ExternalInput")
    if use_rs:
        out_d = nc.dram_tensor("out_shard", [T // NCORES, H], F32,
                               kind="ExternalOutput")
    else:
        out_d = nc.dram_tensor("out_partial", [T, H], F32,
                               kind="ExternalOutput")

    with tile.TileContext(nc) as tc:
        _build_body(nc, tc, x_d, gw_d, cb_d, wg_d, wu_d, wd_d, esel_d, out_d,
                    use_rs, stages)
    nc.compile()
    return nc


def _build_body(nc, tc, x_d, gw_d, cb_d, wg_d, wu_d, wd_d, esel_d, out_d,
                use_rs=True, stages=4):
    from contextlib import ExitStack
    ctx = ExitStack()
    with ctx:
        res = ctx.enter_context(tc.tile_pool(name="resident", bufs=1))
        tpool = ctx.enter_context(tc.tile_pool(name="tmps", bufs=3))
        ps = ctx.enter_context(tc.tile_pool(name="ps", bufs=2, space="PSUM"))
        dram = ctx.enter_context(tc.tile_pool(name="dram", bufs=1, space="DRAM"))

        id_f = res.tile([P, P], F32, tag="idf", name="id_f")
        make_identity(nc, id_f)
        id_b = res.tile([P, P], BF16, tag="idb", name="id_b")
        make_identity(nc, id_b)

        cbb = res.tile([P, E], F32, tag="cbb", name="cbb")
        nc.sync.dma_start(out=cbb, in_=_bcast_ap(cb_d.ap()))
        eselb = []
        for el in range(EPC):
            t = res.tile([P, E], F32, tag=f"eselb{el}", name=f"eselb{el}")
            nc.sync.dma_start(out=t, in_=_bcast_ap(esel_d.ap()[el]))
            eselb.append(t)

        ones = res.tile([P, E], F32, tag="ones", name="ones")
        nc.vector.memset(ones, 1.0)

        xTb = res.tile([P, NKT, T], BF16, tag="xTb", name="xTb")
        accs = []
        for tt in range(NTT):
            a = res.tile([P, H], F32, tag=f"acc{tt}", name=f"acc{tt}")
            nc.vector.memset(a, 0.0)
            accs.append(a)
        coefsel = []
        for tt in range(NTT):
            t = res.tile([P, EPC], F32, tag=f"cs{tt}", name=f"cs{tt}")
            coefsel.append(t)

        # ---------------- stage 0+1: x transpose & routing (fp32) ----------
        with tc.tile_pool(name="route", bufs=1) as rpool, \
             tc.tile_pool(name="routetmp", bufs=2) as rtmp:
            xTf = rpool.tile([P, NKT, T], F32, tag="xTf", name="xTf")
            gwT = rpool.tile([P, NKT, E], F32, tag="gwT", name="gwT")

            for tt in range(NTT):
                xa = rtmp.tile([P, H], F32, tag="xa", name=f"xa{tt}")
                nc.sync.dma_start(out=xa, in_=x_d.ap()[tt * P:(tt + 1) * P, :])
                for j in range(NKT // 4):
                    pst = ps.tile([P, 512], F32, tag="tr", name=f"psx{tt}_{j}")
                    for q in range(4):
                        kt = 4 * j + q
                        nc.tensor.transpose(pst[:, q * P:(q + 1) * P],
                                            xa[:, kt * P:(kt + 1) * P], id_f)
                    sl = (slice(None), slice(4 * j, 4 * j + 4),
                          slice(tt * P, (tt + 1) * P))
                    nc.vector.tensor_copy(
                        xTf[sl], pst.rearrange("p (a b) -> p a b", b=P))
                    nc.scalar.activation(
                        xTb[sl], pst.rearrange("p (a b) -> p a b", b=P), AF.Copy)

            gwa = rpool.tile([E, H], F32, tag="gwa", name="gwa")
            nc.sync.dma_start(out=gwa, in_=gw_d.ap())
            for j in range(NKT // 4):
                pst = ps.tile([P, 64], F32, tag="tr", name=f"psg{j}")
                for q in range(4):
                    kt = 4 * j + q
                    nc.tensor.transpose(pst[:, q * E:(q + 1) * E],
                                        gwa[:, kt * P:(kt + 1) * P],
                                        id_f[:E, :E])
                nc.vector.tensor_copy(
                    gwT[:, 4 * j:4 * j + 4, :],
                    pst.rearrange("p (a b) -> p a b", b=E))

            # routing per token tile
            for tt in range(NTT):
                psl = ps.tile([P, E], F32, tag="mmg", name=f"psl{tt}")
                for kt in range(NKT):
                    nc.tensor.matmul(psl, xTf[:, kt, tt * P:(tt + 1) * P],
                                     gwT[:, kt, :],
                                     start=(kt == 0), stop=(kt == NKT - 1))
                s_t = rtmp.tile([P, E], F32, tag="s_t", name=f"s{tt}")
                nc.scalar.activation(s_t, psl, AF.Sigmoid)
                sfc = rtmp.tile([P, E], F32, tag="sfc", name=f"sfc{tt}")
                nc.vector.tensor_add(sfc, s_t, cbb)
                sfc_g = sfc.rearrange("p (g q) -> p g q", q=E // 4)

                m1 = rtmp.tile([P, 4], F32, tag="m1", name=f"m1{tt}")
                nc.vector.reduce_max(m1, sfc_g, axis=mybir.AxisListType.X)
                eq = rtmp.tile([P, E], F32, tag="eq", name=f"eq{tt}")
                for g in range(4):
                    nc.vector.tensor_scalar(
                        eq[:, 4 * g:4 * g + 4], sfc[:, 4 * g:4 * g + 4],
                        m1[:, g:g + 1], None, OP.is_equal)
                gsm = rtmp.tile([P, E], F32, tag="gsm", name=f"gsm{tt}")
                nc.vector.scalar_tensor_tensor(
                    out=gsm, in0=eq, scalar=-BIG, in1=sfc,
                    op0=OP.mult, op1=OP.add)
                m2 = rtmp.tile([P, 4], F32, tag="m2", name=f"m2{tt}")
                nc.vector.reduce_max(m2, gsm.rearrange("p (g q) -> p g q",
                                                       q=E // 4),
                                     axis=mybir.AxisListType.X)
                gsc = rtmp.tile([P, 4], F32, tag="gsc", name=f"gsc{tt}")
                nc.vector.tensor_add(gsc, m1, m2)

                g1 = rtmp.tile([P, 1], F32, tag="g1", name=f"g1{tt}")
                nc.vector.reduce_max(g1, gsc, axis=mybir.AxisListType.X)
                eqg = rtmp.tile([P, 4], F32, tag="eqg", name=f"eqg{tt}")
                nc.vector.tensor_scalar(eqg, gsc, g1[:, 0:1], None, OP.is_equal)
                gsc2 = rtmp.tile([P, 4], F32, tag="gsc2", name=f"gsc2{tt}")
                nc.vector.scalar_tensor_tensor(
                    out=gsc2, in0=eqg, scalar=-BIG, in1=gsc,
                    op0=OP.mult, op1=OP.add)
                g2 = rtmp.tile([P, 1], F32, tag="g2", name=f"g2{tt}")
                nc.vector.reduce_max(g2, gsc2, axis=mybir.AxisListType.X)
                gmask = rtmp.tile([P, 4], F32, tag="gmask", name=f"gmask{tt}")
                nc.vector.tensor_scalar(gmask, gsc, g2[:, 0:1], None, OP.is_ge)

                emask = rtmp.tile([P, E], F32, tag="emask", name=f"emask{tt}")
                for g in range(4):
                    nc.vector.tensor_scalar(
                        emask[:, 4 * g:4 * g + 4], ones[:, 0:4],
                        gmask[:, g:g + 1], None, OP.mult)
                emneg = rtmp.tile([P, E], F32, tag="emneg", name=f"emneg{tt}")
                nc.vector.tensor_scalar(emneg, emask, 1.0, BIG,
                                        OP.subtract, OP.mult)
                masked = rtmp.tile([P, E], F32, tag="masked", name=f"msk{tt}")
                nc.vector.tensor_tensor(masked, sfc, emask, OP.mult)
                nc.vector.tensor_tensor(masked, masked, emneg, OP.add)

                t8 = rtmp.tile([P, 8], F32, tag="t8", name=f"t8{tt}")
                nc.vector.max(t8, masked)
                selm = rtmp.tile([P, E], F32, tag="selm", name=f"selm{tt}")
                nc.vector.tensor_scalar(selm, masked, t8[:, 5:6], None,
                                        OP.is_ge)
                w16 = rtmp.tile([P, E], F32, tag="w16", name=f"w16{tt}")
                nc.vector.tensor_tensor(w16, s_t, selm, OP.mult)
                wsum = rtmp.tile([P, 1], F32, tag="wsum", name=f"wsum{tt}")
                nc.vector.reduce_sum(wsum, w16, axis=mybir.AxisListType.X)
                winv = rtmp.tile([P, 1], F32, tag="winv", name=f"winv{tt}")
                nc.vector.reciprocal(winv, wsum)
                coef = rtmp.tile([P, E], F32, tag="coef", name=f"coef{tt}")
                nc.vector.tensor_scalar(coef, w16, winv[:, 0:1], RSF,
                                        OP.mult, OP.mult)
                for el in range(EPC):
                    csm = rtmp.tile([P, E], F32, tag=f"csm{el}",
                                    name=f"csm{el}_{tt}")
                    nc.vector.tensor_tensor(csm, coef, eselb[el], OP.mult)
                    nc.vector.reduce_sum(coefsel[tt][:, el:el + 1], csm,
                                         axis=mybir.AxisListType.X)

        # ---------------- stage 2+3: experts ------------------------------
        wpool = ctx.enter_context(tc.tile_pool(name="wstream", bufs=2))
        bpool = ctx.enter_context(tc.tile_pool(name="btiles", bufs=2))
        bdpool = ctx.enter_context(tc.tile_pool(name="bd", bufs=1))
        hpool = ctx.enter_context(tc.tile_pool(name="hh", bufs=2))
        n_exp = EPC if stages >= 4 else (1 if stages >= 2 else 0)
        for e in range(n_exp):
            hh = hpool.tile([P, NIB, T], BF16, tag="hh", name=f"hh{e}")

            for ib in range(NIB):
                ag = wpool.tile([P, H], BF16, tag="ag", name=f"ag{e}_{ib}")
                nc.gpsimd.dma_start(
                    out=ag, in_=wg_d.ap()[e, ib * P:(ib + 1) * P, :])
                au = wpool.tile([P, H], BF16, tag="au", name=f"au{e}_{ib}")
                nc.gpsimd.dma_start(
                    out=au, in_=wu_d.ap()[e, ib * P:(ib + 1) * P, :])

                bg = bpool.tile([P, NKT, P], BF16, tag="bg", name=f"bg{e}_{ib}")
                bu = bpool.tile([P, NKT, P], BF16, tag="bu", name=f"bu{e}_{ib}")
                for src, dst in ((ag, bg), (au, bu)):
                    for j in range(NKT // 4):
                        pst = ps.tile([P, 512], BF16, tag="tr",
                                      name=f"pst{e}_{ib}_{j}")
                        for q in range(4):
                            kt = 4 * j + q
                            nc.tensor.transpose(pst[:, q * P:(q + 1) * P],
                                                src[:, kt * P:(kt + 1) * P],
                                                id_b)
                        nc.vector.tensor_copy(
                            dst[:, 4 * j:4 * j + 4, :],
                            pst.rearrange("p (a b) -> p a b", b=P))

                psg = ps.tile([P, T], F32, tag="mmg", name=f"psg{e}_{ib}")
                psu = ps.tile([P, T], F32, tag="mmu", name=f"psu{e}_{ib}")
                for kt in range(NKT):
                    nc.tensor.matmul(psg, bg[:, kt, :], xTb[:, kt, :],
                                     start=(kt == 0), stop=(kt == NKT - 1))
                for kt in range(NKT):
                    nc.tensor.matmul(psu, bu[:, kt, :], xTb[:, kt, :],
                                     start=(kt == 0), stop=(kt == NKT - 1))
                hsig = tpool.tile([P, T], F32, tag="hsig", name=f"hg{e}_{ib}")
                nc.scalar.activation(hsig, psg, AF.Sigmoid)
                hsil = tpool.tile([P, T], F32, tag="hsil", name=f"hs{e}_{ib}")
                nc.vector.tensor_tensor(hsil, hsig, psg, OP.mult)
                nc.vector.tensor_tensor(hh[:, ib, :], hsil, psu, OP.mult)

            if stages < 3:
                continue
            # wd: [H, I] -> bd[i_win, ib, ht, h_win]
            bd = bdpool.tile([P, NIB, NKT, P], BF16, tag="bd", name=f"bd{e}")
            for ht in range(NKT):
                ad = wpool.tile([P, I], BF16, tag="ad", name=f"ad{e}_{ht}")
                nc.gpsimd.dma_start(
                    out=ad, in_=wd_d.ap()[e, ht * P:(ht + 1) * P, :])
                for j in range((NIB + 3) // 4):
                    nblk = min(4, NIB - 4 * j)
                    pst = ps.tile([P, 512], BF16, tag="tr",
                                  name=f"psd{e}_{ht}_{j}")
                    for q in range(nblk):
                        ib = 4 * j + q
                        nc.tensor.transpose(pst[:, q * P:(q + 1) * P],
                                            ad[:, ib * P:(ib + 1) * P], id_b)
                    nc.vector.tensor_copy(
                        bd[:, 4 * j:4 * j + nblk, ht, :],
                        pst[:, :nblk * P].rearrange("p (a b) -> p a b", b=P))

            for tq in range(NTT):
                for nq in range(H // 512):
                    psy = ps.tile([P, 512], F32, tag="my",
                                  name=f"psy{e}_{tq}_{nq}")
                    for ib in range(NIB):
                        nc.tensor.matmul(
                            psy, hh[:, ib, tq * P:(tq + 1) * P],
                            bd[:, ib, 4 * nq:4 * nq + 4, :],
                            start=(ib == 0), stop=(ib == NIB - 1))
                    nc.vector.scalar_tensor_tensor(
                        out=accs[tq][:, nq * 512:(nq + 1) * 512],
                        in0=psy, scalar=coefsel[tq][:, e:e + 1],
                        in1=accs[tq][:, nq * 512:(nq + 1) * 512],
                        op0=OP.mult, op1=OP.add)

        # ---------------- stage 4: combine across cores --------------------
        if use_rs:
            y_full = dram.tile([T, H], F32, name="y_full")
            y_rs = dram.tile([T // NCORES, H], F32, name="y_rs")
            for tt in range(NTT):
                nc.sync.dma_start(out=y_full[tt * P:(tt + 1) * P, :],
                                  in_=accs[tt])
            nc.gpsimd.collective_compute(
                "ReduceScatter", OP.add,
                replica_groups=[list(range(NCORES))],
                ins=[y_full.opt()], outs=[y_rs.opt()])
            nc.sync.dma_start(out=out_d.ap(), in_=y_rs[:, :])
        else:
            for tt in range(NTT):
                nc.sync.dma_start(out=out_d.ap()[tt * P:(tt + 1) * P, :],
                                  in_=accs[tt])


_NC_CACHE = {}


def _get_nc(use_rs=True, stages=4, ncores=NCORES):
    key = (use_rs, stages, ncores)
    if key not in _NC_CACHE:
        _NC_CACHE[key] = build_nc(use_rs, stages, ncores)
    return _NC_CACHE[key]


def _in_maps(inputs):
    x = np.ascontiguousarray(inputs["hidden_states"], dtype=np.float32)
    gw = np.ascontiguousarray(inputs["gate_weight"], dtype=np.float32)
    cb = np.ascontiguousarray(inputs["correction_bias"], dtype=np.float32)
    wg = np.ascontiguousarray(inputs["w_gate"], dtype=np.float32)
    wu = np.ascontiguousarray(inputs["w_up"], dtype=np.float32)
    wd = np.ascontiguousarray(inputs["w_down"], dtype=np.float32)
    maps = []
    for c in range(NCORES):
        esel = np.zeros((EPC, E), np.float32)
        for el in range(EPC):
            esel[el, c * EPC + el] = 1.0
        maps.append({
            "x": x, "gw": gw, "cb": cb,
            "wg": np.ascontiguousarray(wg[c * EPC:(c + 1) * EPC]),
            "wu": np.ascontiguousarray(wu[c * EPC:(c + 1) * EPC]),
            "wd": np.ascontiguousarray(wd[c * EPC:(c + 1) * EPC]),
            "esel": esel,
        })
    return maps


def run(inputs, trace=False, use_rs=True, stages=4, ncores=NCORES):
    nc = _get_nc(use_rs, stages, ncores)
    res = run_bass_kernel_spmd(nc, _in_maps(inputs)[:ncores],
                               core_ids=list(range(ncores)), trace=trace)
    if use_rs:
        out = np.concatenate(
            [res.results[c]["out_shard"] for c in range(ncores)], axis=0)
    else:
        out = np.sum([res.results[c]["out_partial"] for c in range(ncores)],
                     axis=0)
    return out, res


def kernel(**inputs) -> np.ndarray:
    out, _ = run(inputs)
    return out



# revision 3
# speedup vs baseline: 1.3329x; 1.3329x over previous
"""DeepseekV2 MoE (T=512, H=2048, I=1408, E=16, top-6 group-limited routing)
on 8 trn2 NeuronCores, expert-parallel (2 experts/core) with on-device fp32
routing, bf16 expert GEMMs, and a ReduceScatter combine.

Self-contained: hardcodes all shapes; builds one SPMD Bass program shared by
all 8 cores. Per-core inputs carry that core's expert weight slices already in
bf16 and pre-arranged so the contraction dim lands on SBUF partitions (layout
prep is host-side sharding work; all arithmetic on the activations — routing,
GEMMs, combine — runs on device):
  wgt/wut[e, ib, hh, kt, ii] = w[e, ib*128+ii, kt*128+hh]   (4KB/partition DMA)
  wdt[e, ib, ii, h]          = wd[e, h, ib*128+ii]
The routed_scaling_factor is folded into the per-core expert selector.
"""

import numpy as np

import concourse.bass as bass
import concourse.mybir as mybir
import concourse.tile as tile
from concourse import bacc
from concourse.bass_utils import run_bass_kernel_spmd
from concourse.masks import make_identity

F32 = mybir.dt.float32
BF16 = mybir.dt.bfloat16
AF = mybir.ActivationFunctionType
OP = mybir.AluOpType
AX = mybir.AxisListType

T, H, I, E = 512, 2048, 1408, 16
P = 128
NCORES = 8
NKT = H // P               # 16 k-tiles over H
NIB = I // P               # 11 i-tiles over I
NTT = T // P               # 4 token tiles
NHQ = H // 512             # 4 output column chunks
RSF = 2.5
BIG = 1.0e30


def _bcast_part(ap, parts=P):
    """Partition-broadcast a 1D AP to [parts, n]."""
    return bass.AP(tensor=ap.tensor, offset=ap.offset, ap=[[0, parts]] + list(ap.ap))


def _bfree(ap, n):
    """Append an innermost stride-0 (broadcast) free dim of size n."""
    return bass.AP(tensor=ap.tensor, offset=ap.offset, ap=list(ap.ap) + [[0, n]])


def _bmid(ap, n):
    """Insert a stride-0 (broadcast) free dim right after the partition dim."""
    a = list(ap.ap)
    return bass.AP(tensor=ap.tensor, offset=ap.offset, ap=[a[0], [0, n]] + a[1:])


def build_nc(use_rs=True, stages=4, ncores=NCORES):
    epc = E // ncores
    nc = bacc.Bacc("TRN2", target_bir_lowering=False, debug=False,
                   num_devices=ncores)

    x_d = nc.dram_tensor("x", [T, H], F32, kind="ExternalInput")
    gw_d = nc.dram_tensor("gw", [E, H], F32, kind="ExternalInput")
    cb_d = nc.dram_tensor("cb", [E], F32, kind="ExternalInput")
    esel_d = nc.dram_tensor("esel", [epc, E], F32, kind="ExternalInput")
    wgt_d = nc.dram_tensor("wgt", [epc, NIB, P, NKT, P], BF16,
                           kind="ExternalInput")
    wut_d = nc.dram_tensor("wut", [epc, NIB, P, NKT, P], BF16,
                           kind="ExternalInput")
    wdt_d = nc.dram_tensor("wdt", [epc, NIB, P, H], BF16,
                           kind="ExternalInput")
    if use_rs:
        out_d = nc.dram_tensor("out_shard", [T // ncores, H], F32,
                               kind="ExternalOutput")
    else:
        out_d = nc.dram_tensor("out_partial", [T, H], F32,
                               kind="ExternalOutput")

    with tile.TileContext(nc) as tc:
        _build_body(nc, tc, x_d, gw_d, cb_d, esel_d, wgt_d, wut_d, wdt_d,
                    out_d, use_rs, stages, epc, ncores)
    nc.compile()
    return nc


def _build_body(nc, tc, x_d, gw_d, cb_d, esel_d, wgt_d, wut_d, wdt_d, out_d,
                use_rs, stages, epc, ncores):
    from contextlib import ExitStack
    ctx = ExitStack()
    with ctx:
        res = ctx.enter_context(tc.tile_pool(name="resident", bufs=1))
        tmps = ctx.enter_context(tc.tile_pool(name="tmps", bufs=3))
        ps = ctx.enter_context(tc.tile_pool(name="ps", bufs=2, space="PSUM"))
        dram = ctx.enter_context(tc.tile_pool(name="dram", bufs=1,
                                              space="DRAM"))

        id_f = res.tile([P, P], F32, tag="idf", name="id_f")
        make_identity(nc, id_f)

        cbb4 = res.tile([P, NTT, E], F32, tag="cbb4", name="cbb4")
        for tt in range(NTT):
            nc.gpsimd.dma_start(out=cbb4[:, tt, :], in_=_bcast_part(cb_d.ap()))
        eselb = []
        for el in range(epc):
            t = res.tile([P, E], F32, tag=f"eselb{el}", name=f"eselb{el}")
            nc.gpsimd.dma_start(out=t, in_=_bcast_part(esel_d.ap()[el]))
            eselb.append(t)

        xTb = res.tile([P, NKT, T], BF16, tag="xTb", name="xTb")
        accs = []
        for tt in range(NTT):
            a = res.tile([P, H], F32, tag=f"acc{tt}", name=f"acc{tt}")
            nc.vector.memset(a, 0.0)
            accs.append(a)
        # coefsel[e][p, tt] — combine coefficient of this core's expert e for
        # token (tt*128+p), already scaled by RSF (host-folded into esel).
        coefsel = [res.tile([P, NTT], F32, tag=f"cs{el}", name=f"cs{el}")
                   for el in range(epc)]

        # ---------------- stage 0+1: x transpose & routing (fp32) ----------
        with tc.tile_pool(name="route", bufs=1) as rpool, \
             tc.tile_pool(name="routetmp", bufs=2) as rtmp:
            xTf = rpool.tile([P, NKT, T], F32, tag="xTf", name="xTf")
            gwT = rpool.tile([P, NKT, E], F32, tag="gwT", name="gwT")

            for tt in range(NTT):
                xa = rtmp.tile([P, H], F32, tag="xa", name=f"xa{tt}")
                nc.sync.dma_start(out=xa, in_=x_d.ap()[tt * P:(tt + 1) * P, :])
                for j in range(NKT // 4):
                    pst = ps.tile([P, 512], F32, tag="tr", name=f"psx{tt}_{j}")
                    for q in range(4):
                        kt = 4 * j + q
                        nc.tensor.transpose(pst[:, q * P:(q + 1) * P],
                                            xa[:, kt * P:(kt + 1) * P], id_f)
                    sl = (slice(None), slice(4 * j, 4 * j + 4),
                          slice(tt * P, (tt + 1) * P))
                    nc.vector.tensor_copy(
                        xTf[sl], pst.rearrange("p (a b) -> p a b", b=P))
                    nc.scalar.activation(
                        xTb[sl], pst.rearrange("p (a b) -> p a b", b=P),
                        AF.Copy)

            gwa = rpool.tile([E, H], F32, tag="gwa", name="gwa")
            nc.sync.dma_start(out=gwa, in_=gw_d.ap())
            for j in range(NKT // 4):
                pst = ps.tile([P, 64], F32, tag="tr", name=f"psg{j}")
                for q in range(4):
                    kt = 4 * j + q
                    nc.tensor.transpose(pst[:, q * E:(q + 1) * E],
                                        gwa[:, kt * P:(kt + 1) * P],
                                        id_f[:E, :E])
                nc.vector.tensor_copy(
                    gwT[:, 4 * j:4 * j + 4, :],
                    pst.rearrange("p (a b) -> p a b", b=E))

            # router logits + sigmoid, all 4 token tiles into s4
            s4 = rpool.tile([P, NTT, E], F32, tag="s4", name="s4")
            for tt in range(NTT):
                psl = ps.tile([P, E], F32, tag="y", name=f"psl{tt}")
                for kt in range(NKT):
                    nc.tensor.matmul(psl, xTf[:, kt, tt * P:(tt + 1) * P],
                                     gwT[:, kt, :],
                                     start=(kt == 0), stop=(kt == NKT - 1))
                nc.scalar.activation(s4[:, tt, :], psl, AF.Sigmoid)

            # noaux_tc grouped top-k, batched over the 4 token tiles.
            G = 4
            EG = E // G  # 4 experts per group
            sfc = rtmp.tile([P, NTT, E], F32, tag="sfc", name="sfc")
            nc.vector.tensor_add(sfc, s4, cbb4)
            sfc_g = sfc.rearrange("p t (g q) -> p (t g) q", q=EG)

            m1 = rtmp.tile([P, NTT * G], F32, tag="m1", name="m1")
            nc.vector.reduce_max(m1, sfc_g, axis=AX.X)
            eq = rtmp.tile([P, NTT, E], F32, tag="eq", name="eq")
            nc.vector.tensor_tensor(
                eq.rearrange("p t (g q) -> p (t g) q", q=EG), sfc_g,
                _bfree(m1, EG), OP.is_equal)
            gsm = rtmp.tile([P, NTT, E], F32, tag="gsm", name="gsm")
            nc.vector.scalar_tensor_tensor(
                out=gsm, in0=eq, scalar=-BIG, in1=sfc,
                op0=OP.mult, op1=OP.add)
            m2 = rtmp.tile([P, NTT * G], F32, tag="m2", name="m2")
            nc.vector.reduce_max(
                m2, gsm.rearrange("p t (g q) -> p (t g) q", q=EG), axis=AX.X)
            gsc = rtmp.tile([P, NTT * G], F32, tag="gsc", name="gsc")
            nc.vector.tensor_add(gsc, m1, m2)

            g1 = rtmp.tile([P, NTT], F32, tag="g1", name="g1")
            nc.vector.reduce_max(
                g1, gsc.rearrange("p (t g) -> p t g", g=G), axis=AX.X)
            eqg = rtmp.tile([P, NTT * G], F32, tag="eqg", name="eqg")
            nc.vector.tensor_tensor(
                eqg.rearrange("p (t g) -> p t g", g=G),
                gsc.rearrange("p (t g) -> p t g", g=G),
                _bfree(g1, G), OP.is_equal)
            gsc2 = rtmp.tile([P, NTT * G], F32, tag="gsc2", name="gsc2")
            nc.vector.scalar_tensor_tensor(
                out=gsc2, in0=eqg, scalar=-BIG, in1=gsc,
                op0=OP.mult, op1=OP.add)
            g2 = rtmp.tile([P, NTT], F32, tag="g2", name="g2")
            nc.vector.reduce_max(
                g2, gsc2.rearrange("p (t g) -> p t g", g=G), axis=AX.X)
            gmask = rtmp.tile([P, NTT * G], F32, tag="gmask", name="gmask")
            nc.vector.tensor_tensor(
                gmask.rearrange("p (t g) -> p t g", g=G),
                gsc.rearrange("p (t g) -> p t g", g=G),
                _bfree(g2, G), OP.is_ge)

            emask = rtmp.tile([P, NTT, E], F32, tag="emask", name="emask")
            nc.vector.tensor_copy(
                emask.rearrange("p t (g q) -> p (t g) q", q=EG),
                _bfree(gmask, EG))
            emneg = rtmp.tile([P, NTT, E], F32, tag="emneg", name="emneg")
            nc.vector.tensor_scalar(emneg, emask, 1.0, BIG,
                                    OP.subtract, OP.mult)
            masked = rtmp.tile([P, NTT, E], F32, tag="masked", name="masked")
            nc.vector.tensor_tensor(masked, sfc, emask, OP.mult)
            nc.vector.tensor_tensor(masked, masked, emneg, OP.add)

            t8 = rtmp.tile([P, NTT, 8], F32, tag="t8", name="t8")
            for tt in range(NTT):
                nc.vector.max(t8[:, tt, :], masked[:, tt, :])
            selm = rtmp.tile([P, NTT, E], F32, tag="selm", name="selm")
            nc.vector.tensor_tensor(selm, masked, _bfree(t8[:, :, 5], E),
                                    OP.is_ge)
            w16 = rtmp.tile([P, NTT, E], F32, tag="w16", name="w16")
            nc.vector.tensor_tensor(w16, s4, selm, OP.mult)
            wsum = rtmp.tile([P, NTT], F32, tag="wsum", name="wsum")
            nc.vector.reduce_sum(wsum, w16, axis=AX.X)
            winv = rtmp.tile([P, NTT], F32, tag="winv", name="winv")
            nc.vector.reciprocal(winv, wsum)
            coef = rtmp.tile([P, NTT, E], F32, tag="coef", name="coef")
            nc.vector.tensor_tensor(coef, w16, _bfree(winv, E), OP.mult)
            for el in range(epc):
                csm = rtmp.tile([P, NTT, E], F32, tag=f"csm{el}",
                                name=f"csm{el}")
                nc.vector.tensor_tensor(csm, coef, _bmid(eselb[el], NTT),
                                        OP.mult)
                nc.vector.reduce_sum(coefsel[el], csm, axis=AX.X)

        # ---------------- stage 2+3: experts ------------------------------
        wpool = ctx.enter_context(tc.tile_pool(name="wstream", bufs=2))
        bdpool = ctx.enter_context(tc.tile_pool(name="bd", bufs=1))
        hpool = ctx.enter_context(tc.tile_pool(name="hh", bufs=2))
        n_exp = epc if stages >= 4 else (1 if stages >= 2 else 0)
        for e in range(n_exp):
            hh = hpool.tile([P, NIB, T], BF16, tag="hh", name=f"hh{e}")

            for ib in range(NIB):
                ag = wpool.tile([P, NKT, P], BF16, tag="ag", name=f"ag{e}_{ib}")
                nc.sync.dma_start(out=ag, in_=wgt_d.ap()[e, ib])
                au = wpool.tile([P, NKT, P], BF16, tag="au", name=f"au{e}_{ib}")
                nc.sync.dma_start(out=au, in_=wut_d.ap()[e, ib])

                psg = ps.tile([P, T], F32, tag="g", name=f"psg{e}_{ib}")
                psu = ps.tile([P, T], F32, tag="u", name=f"psu{e}_{ib}")
                for kt in range(NKT):
                    nc.tensor.matmul(psg, ag[:, kt, :], xTb[:, kt, :],
                                     start=(kt == 0), stop=(kt == NKT - 1))
                for kt in range(NKT):
                    nc.tensor.matmul(psu, au[:, kt, :], xTb[:, kt, :],
                                     start=(kt == 0), stop=(kt == NKT - 1))
                hsig = tmps.tile([P, T], F32, tag="hsig", name=f"hg{e}_{ib}")
                nc.scalar.activation(hsig, psg, AF.Sigmoid)
                hsil = tmps.tile([P, T], F32, tag="hsil", name=f"hs{e}_{ib}")
                nc.vector.tensor_tensor(hsil, hsig, psg, OP.mult)
                nc.vector.tensor_tensor(hh[:, ib, :], hsil, psu, OP.mult)

            if stages < 3:
                continue
            bd = bdpool.tile([P, NIB, H], BF16, tag="bd", name=f"bd{e}")
            for ib in range(NIB):
                nc.sync.dma_start(out=bd[:, ib, :], in_=wdt_d.ap()[e, ib])

            for tq in range(NTT):
                for nq in range(NHQ):
                    psy = ps.tile([P, 512], F32, tag="y",
                                  name=f"psy{e}_{tq}_{nq}")
                    for ib in range(NIB):
                        nc.tensor.matmul(
                            psy, hh[:, ib, tq * P:(tq + 1) * P],
                            bd[:, ib, nq * 512:(nq + 1) * 512],
                            start=(ib == 0), stop=(ib == NIB - 1))
                    nc.vector.scalar_tensor_tensor(
                        out=accs[tq][:, nq * 512:(nq + 1) * 512],
                        in0=psy, scalar=coefsel[e][:, tq:tq + 1],
                        in1=accs[tq][:, nq * 512:(nq + 1) * 512],
                        op0=OP.mult, op1=OP.add)

        # ---------------- stage 4: combine across cores --------------------
        if use_rs:
            y_full = dram.tile([T, H], F32, name="y_full")
            y_rs = dram.tile([T // ncores, H], F32, name="y_rs")
            for tt in range(NTT):
                nc.sync.dma_start(out=y_full[tt * P:(tt + 1) * P, :],
                                  in_=accs[tt])
            nc.gpsimd.collective_compute(
                "ReduceScatter", OP.add,
                replica_groups=[list(range(ncores))],
                ins=[y_full.opt()], outs=[y_rs.opt()])
            nc.sync.dma_start(out=out_d.ap(), in_=y_rs[:, :])
        else:
            for tt in range(NTT):
                nc.sync.dma_start(out=out_d.ap()[tt * P:(tt + 1) * P, :],
                                  in_=accs[tt])


_NC_CACHE = {}


def _get_nc(use_rs=True, stages=4, ncores=NCORES):
    key = (use_rs, stages, ncores)
    if key not in _NC_CACHE:
        _NC_CACHE[key] = build_nc(use_rs, stages, ncores)
    return _NC_CACHE[key]


def _in_maps(inputs, ncores=NCORES):
    import ml_dtypes
    bf16 = ml_dtypes.bfloat16
    epc = E // ncores
    x = np.ascontiguousarray(inputs["hidden_states"], dtype=np.float32)
    gw = np.ascontiguousarray(inputs["gate_weight"], dtype=np.float32)
    cb = np.ascontiguousarray(inputs["correction_bias"], dtype=np.float32)
    wg = np.asarray(inputs["w_gate"], dtype=np.float32).astype(bf16)
    wu = np.asarray(inputs["w_up"], dtype=np.float32).astype(bf16)
    wd = np.asarray(inputs["w_down"], dtype=np.float32).astype(bf16)
    maps = []
    for c in range(ncores):
        sl = slice(c * epc, (c + 1) * epc)
        # [e, i, h] -> [e, ib, hh, kt, ii]
        wgt = np.ascontiguousarray(
            wg[sl].reshape(epc, NIB, P, NKT, P).transpose(0, 1, 4, 3, 2))
        wut = np.ascontiguousarray(
            wu[sl].reshape(epc, NIB, P, NKT, P).transpose(0, 1, 4, 3, 2))
        # [e, h, i] -> [e, ib, ii, h]
        wdt = np.ascontiguousarray(
            wd[sl].reshape(epc, H, NIB, P).transpose(0, 2, 3, 1))
        esel = np.zeros((epc, E), np.float32)
        for el in range(epc):
            esel[el, c * epc + el] = RSF
        maps.append({
            "x": x, "gw": gw, "cb": cb, "esel": esel,
            "wgt": wgt, "wut": wut, "wdt": wdt,
        })
    return maps


def run(inputs, trace=False, use_rs=True, stages=4, ncores=NCORES):
    nc = _get_nc(use_rs, stages, ncores)
    res = run_bass_kernel_spmd(nc, _in_maps(inputs, ncores),
                               core_ids=list(range(ncores)), trace=trace)
    if use_rs:
        out = np.concatenate(
            [np.asarray(res.results[c]["out_shard"], dtype=np.float32)
             for c in range(ncores)], axis=0)
    else:
        out = np.sum([res.results[c]["out_partial"] for c in range(ncores)],
                     axis=0).astype(np.float32)
    return out, res


def kernel(**inputs) -> np.ndarray:
    out, _ = run(inputs)
    return out


# revision 10
# speedup vs baseline: 1.4617x; 1.0966x over previous
"""DeepseekV2 MoE (T=512, H=2048, I=1408, E=16, top-6 group-limited routing)
on 8 trn2 NeuronCores, expert-parallel (2 experts/core) with on-device fp32
routing, bf16 expert GEMMs, and a ReduceScatter combine.

Self-contained: hardcodes all shapes; builds one SPMD Bass program shared by
all 8 cores. Per-core inputs carry that core's expert weight slices already in
bf16 and pre-arranged so the contraction dim lands on SBUF partitions (layout
prep is host-side sharding work; all arithmetic on the activations — routing,
GEMMs, combine — runs on device):
  wgt/wut[e, ib, hh, kt, ii] = w[e, ib*128+ii, kt*128+hh]   (4KB/partition DMA)
  wdt[e, ib, ii, h]          = wd[e, h, ib*128+ii]
The routed_scaling_factor is folded into the per-core expert selector.
"""

import numpy as np

import concourse.bass as bass
import concourse.mybir as mybir
import concourse.tile as tile
from concourse import bacc
from concourse.bass_utils import run_bass_kernel_spmd
from concourse.masks import make_identity

F32 = mybir.dt.float32
BF16 = mybir.dt.bfloat16
AF = mybir.ActivationFunctionType
OP = mybir.AluOpType
AX = mybir.AxisListType

T, H, I, E = 512, 2048, 1408, 16
P = 128
NCORES = 8
NKT = H // P               # 16 k-tiles over H
NIB = I // P               # 11 i-tiles over I
NTT = T // P               # 4 token tiles
NHQ = H // 512             # 4 output column chunks
RSF = 2.5
BIG = 1.0e30
CAP = 256                  # token capacity per expert (mean load 192, +5.7σ)
NCT = CAP // P             # 2 capacity tiles
TRASH = 480.0              # scatter row for unselected tokens (>= CAP)
I32 = mybir.dt.int32


def _bcast_part(ap, parts=P):
    """Partition-broadcast a 1D AP to [parts, n]."""
    return bass.AP(tensor=ap.tensor, offset=ap.offset, ap=[[0, parts]] + list(ap.ap))


def _bfree(ap, n):
    """Append an innermost stride-0 (broadcast) free dim of size n."""
    return bass.AP(tensor=ap.tensor, offset=ap.offset, ap=list(ap.ap) + [[0, n]])


def _bmid(ap, n):
    """Insert a stride-0 (broadcast) free dim right after the partition dim."""
    a = list(ap.ap)
    return bass.AP(tensor=ap.tensor, offset=ap.offset, ap=[a[0], [0, n]] + a[1:])


def build_nc(use_rs=True, stages=4, ncores=NCORES):
    epc = E // ncores
    nc = bacc.Bacc("TRN2", target_bir_lowering=False, debug=False,
                   num_devices=ncores)

    x_d = nc.dram_tensor("x", [T, H], F32, kind="ExternalInput")
    gw_d = nc.dram_tensor("gw", [E, H], F32, kind="ExternalInput")
    cb_d = nc.dram_tensor("cb", [E], F32, kind="ExternalInput")
    esel_d = nc.dram_tensor("esel", [epc, E], F32, kind="ExternalInput")
    wgt_d = nc.dram_tensor("wgt", [epc, NIB, P, NKT, P], BF16,
                           kind="ExternalInput")
    wut_d = nc.dram_tensor("wut", [epc, NIB, P, NKT, P], BF16,
                           kind="ExternalInput")
    wdt_d = nc.dram_tensor("wdt", [epc, NIB, P, H], BF16,
                           kind="ExternalInput")
    if use_rs:
        # out_shard[tq, r, :] = combined output for token tq*128 + rank*16 + r
        out_d = nc.dram_tensor("out_shard", [NTT, P // ncores, H], F32,
                               kind="ExternalOutput")
    else:
        out_d = nc.dram_tensor("out_partial", [T, H], F32,
                               kind="ExternalOutput")

    with tile.TileContext(nc) as tc:
        _build_body(nc, tc, x_d, gw_d, cb_d, esel_d, wgt_d, wut_d, wdt_d,
                    out_d, use_rs, stages, epc, ncores)
    nc.compile()
    return nc


def _build_body(nc, tc, x_d, gw_d, cb_d, esel_d, wgt_d, wut_d, wdt_d, out_d,
                use_rs, stages, epc, ncores):
    from contextlib import ExitStack
    ctx = ExitStack()
    with ctx:
        res = ctx.enter_context(tc.tile_pool(name="resident", bufs=1))
        tmps = ctx.enter_context(tc.tile_pool(name="tmps", bufs=3))
        ps = ctx.enter_context(tc.tile_pool(name="ps", bufs=2, space="PSUM"))
        dram = ctx.enter_context(tc.tile_pool(name="dram", bufs=1,
                                              space="DRAM"))

        id_f = res.tile([P, P], F32, tag="idf", name="id_f")
        make_identity(nc, id_f)

        cbb4 = res.tile([P, NTT, E], F32, tag="cbb4", name="cbb4")
        for tt in range(NTT):
            nc.gpsimd.dma_start(out=cbb4[:, tt, :], in_=_bcast_part(cb_d.ap()))
        eselb = []
        for el in range(epc):
            t = res.tile([P, E], F32, tag=f"eselb{el}", name=f"eselb{el}")
            nc.gpsimd.dma_start(out=t, in_=_bcast_part(esel_d.ap()[el]))
            eselb.append(t)

        xTb = res.tile([P, NKT, T], BF16, tag="xTb", name="xTb")
        accs = []
        for tt in range(NTT):
            a = res.tile([P, H], F32, tag=f"acc{tt}", name=f"acc{tt}")
            nc.vector.memset(a, 0.0)
            accs.append(a)
        # coefsel[e][p, tt] — combine coefficient of this core's expert e for
        # token (tt*128+p), already scaled by RSF (host-folded into esel).
        coefsel = [res.tile([P, NTT], F32, tag=f"cs{el}", name=f"cs{el}")
                   for el in range(epc)]

        _route_stage(nc, tc, ps, x_d, gw_d, cbb4, eselb, coefsel, epc,
                     id_f, xTb)
            gwT = rpool.tile([P, NKT, E], F32, tag="gwT", name="gwT")

            for tt in range(NTT):
                xa = rtmp.tile([P, H], F32, tag="xa", name=f"xa{tt}")
                nc.gpsimd.dma_start(out=xa,
                                    in_=x_d.ap()[tt * P:(tt + 1) * P, :])
                for j in range(NKT // 4):
                    pst = ps.tile([P, 512], F32, tag="tr", name=f"psx{tt}_{j}")
                    for q in range(4):
                        kt = 4 * j + q
                        nc.tensor.transpose(pst[:, q * P:(q + 1) * P],
                                            xa[:, kt * P:(kt + 1) * P], id_f)
                    sl = (slice(None), slice(4 * j, 4 * j + 4),
                          slice(tt * P, (tt + 1) * P))
                    nc.vector.tensor_copy(
                        xTf[sl], pst.rearrange("p (a b) -> p a b", b=P))
                    nc.scalar.activation(
                        xTb[sl], pst.rearrange("p (a b) -> p a b", b=P),
                        AF.Copy)

            gwa = rpool.tile([E, H], F32, tag="gwa", name="gwa")
            nc.gpsimd.dma_start(out=gwa, in_=gw_d.ap())
            for j in range(NKT // 4):
                pst = ps.tile([P, 64], F32, tag="tr", name=f"psg{j}")
                for q in range(4):
                    kt = 4 * j + q
                    nc.tensor.transpose(pst[:, q * E:(q + 1) * E],
                                        gwa[:, kt * P:(kt + 1) * P],
                                        id_f[:E, :E])
                nc.vector.tensor_copy(
                    gwT[:, 4 * j:4 * j + 4, :],
                    pst.rearrange("p (a b) -> p a b", b=E))

            # router logits + sigmoid, all 4 token tiles into s4
            s4 = rpool.tile([P, NTT, E], F32, tag="s4", name="s4")
            for tt in range(NTT):
                psl = ps.tile([P, E], F32, tag="y", name=f"psl{tt}")
                for kt in range(NKT):
                    nc.tensor.matmul(psl, xTf[:, kt, tt * P:(tt + 1) * P],
                                     gwT[:, kt, :],
                                     start=(kt == 0), stop=(kt == NKT - 1))
                nc.scalar.activation(s4[:, tt, :], psl, AF.Sigmoid)

            # noaux_tc grouped top-k, batched over the 4 token tiles.
            G = 4
            EG = E // G  # 4 experts per group
            sfc = rtmp.tile([P, NTT, E], F32, tag="sfc", name="sfc")
            nc.vector.tensor_add(sfc, s4, cbb4)
            sfc_g = sfc.rearrange("p t (g q) -> p (t g) q", q=EG)

            m1 = rtmp.tile([P, NTT * G], F32, tag="m1", name="m1")
            nc.vector.reduce_max(m1, sfc_g, axis=AX.X)
            eq = rtmp.tile([P, NTT, E], F32, tag="eq", name="eq")
            nc.vector.tensor_tensor(
                eq.rearrange("p t (g q) -> p (t g) q", q=EG), sfc_g,
                _bfree(m1, EG), OP.is_equal)
            gsm = rtmp.tile([P, NTT, E], F32, tag="gsm", name="gsm")
            nc.vector.scalar_tensor_tensor(
                out=gsm, in0=eq, scalar=-BIG, in1=sfc,
                op0=OP.mult, op1=OP.add)
            m2 = rtmp.tile([P, NTT * G], F32, tag="m2", name="m2")
            nc.vector.reduce_max(
                m2, gsm.rearrange("p t (g q) -> p (t g) q", q=EG), axis=AX.X)
            gsc = rtmp.tile([P, NTT * G], F32, tag="gsc", name="gsc")
            nc.vector.tensor_add(gsc, m1, m2)

            g1 = rtmp.tile([P, NTT], F32, tag="g1", name="g1")
            nc.vector.reduce_max(
                g1, gsc.rearrange("p (t g) -> p t g", g=G), axis=AX.X)
            eqg = rtmp.tile([P, NTT * G], F32, tag="eqg", name="eqg")
            nc.vector.tensor_tensor(
                eqg.rearrange("p (t g) -> p t g", g=G),
                gsc.rearrange("p (t g) -> p t g", g=G),
                _bfree(g1, G), OP.is_equal)
            gsc2 = rtmp.tile([P, NTT * G], F32, tag="gsc2", name="gsc2")
            nc.vector.scalar_tensor_tensor(
                out=gsc2, in0=eqg, scalar=-BIG, in1=gsc,
                op0=OP.mult, op1=OP.add)
            g2 = rtmp.tile([P, NTT], F32, tag="g2", name="g2")
            nc.vector.reduce_max(
                g2, gsc2.rearrange("p (t g) -> p t g", g=G), axis=AX.X)
            gmask = rtmp.tile([P, NTT * G], F32, tag="gmask", name="gmask")
            nc.vector.tensor_tensor(
                gmask.rearrange("p (t g) -> p t g", g=G),
                gsc.rearrange("p (t g) -> p t g", g=G),
                _bfree(g2, G), OP.is_ge)

            emask = rtmp.tile([P, NTT, E], F32, tag="emask", name="emask")
            nc.vector.tensor_copy(
                emask.rearrange("p t (g q) -> p (t g) q", q=EG),
                _bfree(gmask, EG))
            emneg = rtmp.tile([P, NTT, E], F32, tag="emneg", name="emneg")
            nc.vector.tensor_scalar(emneg, emask, 1.0, BIG,
                                    OP.subtract, OP.mult)
            masked = rtmp.tile([P, NTT, E], F32, tag="masked", name="masked")
            nc.vector.tensor_tensor(masked, sfc, emask, OP.mult)
            nc.vector.tensor_tensor(masked, masked, emneg, OP.add)

            t8 = rtmp.tile([P, NTT, 8], F32, tag="t8", name="t8")
            for tt in range(NTT):
                nc.vector.max(t8[:, tt, :], masked[:, tt, :])
            selm = rtmp.tile([P, NTT, E], F32, tag="selm", name="selm")
            nc.vector.tensor_tensor(selm, masked, _bfree(t8[:, :, 5], E),
                                    OP.is_ge)
            w16 = rtmp.tile([P, NTT, E], F32, tag="w16", name="w16")
            nc.vector.tensor_tensor(w16, s4, selm, OP.mult)
            wsum = rtmp.tile([P, NTT], F32, tag="wsum", name="wsum")
            nc.vector.reduce_sum(wsum, w16, axis=AX.X)
            winv = rtmp.tile([P, NTT], F32, tag="winv", name="winv")
            nc.vector.reciprocal(winv, wsum)
            coef = rtmp.tile([P, NTT, E], F32, tag="coef", name="coef")
            nc.vector.tensor_tensor(coef, w16, _bfree(winv, E), OP.mult)
            for el in range(epc):
                csm = rtmp.tile([P, NTT, E], F32, tag=f"csm{el}",
                                name=f"csm{el}")
                nc.vector.tensor_tensor(csm, coef, _bmid(eselb[el], NTT),
                                        OP.mult)
                nc.vector.reduce_sum(coefsel[el], csm, axis=AX.X)

        # ---------------- stage 2: per-expert up/gate GEMMs ----------------
        wpool = ctx.enter_context(tc.tile_pool(name="wstream", bufs=2))
        bdpool = ctx.enter_context(tc.tile_pool(name="bd", bufs=2))
        hpool = ctx.enter_context(tc.tile_pool(name="hh", bufs=2))
        n_exp = epc if stages >= 4 else (1 if stages >= 2 else 0)
        hhs, bds = [], []
        for e in range(n_exp):
            hh = hpool.tile([P, NIB, T], BF16, tag="hh", name=f"hh{e}")
            hhs.append(hh)

            for ib in range(NIB):
                ag = wpool.tile([P, NKT, P], BF16, tag="ag", name=f"ag{e}_{ib}")
                nc.sync.dma_start(out=ag, in_=wgt_d.ap()[e, ib])
                au = wpool.tile([P, NKT, P], BF16, tag="au", name=f"au{e}_{ib}")
                nc.sync.dma_start(out=au, in_=wut_d.ap()[e, ib])

                psg = ps.tile([P, T], F32, tag="g", name=f"psg{e}_{ib}")
                psu = ps.tile([P, T], F32, tag="u", name=f"psu{e}_{ib}")
                for kt in range(NKT):
                    nc.tensor.matmul(psg, ag[:, kt, :], xTb[:, kt, :],
                                     start=(kt == 0), stop=(kt == NKT - 1))
                for kt in range(NKT):
                    nc.tensor.matmul(psu, au[:, kt, :], xTb[:, kt, :],
                                     start=(kt == 0), stop=(kt == NKT - 1))
                hsig = tmps.tile([P, T], F32, tag="hsig", name=f"hg{e}_{ib}")
                nc.scalar.activation(hsig, psg, AF.Sigmoid)
                hsil = tmps.tile([P, T], F32, tag="hsil", name=f"hs{e}_{ib}")
                nc.vector.tensor_tensor(hsil, hsig, psg, OP.mult)
                nc.vector.tensor_tensor(hh[:, ib, :], hsil, psu, OP.mult)

            bd = bdpool.tile([P, NIB, H], BF16, tag="bd", name=f"bd{e}")
            bds.append(bd)
            for ib in range(NIB):
                nc.sync.dma_start(out=bd[:, ib, :], in_=wdt_d.ap()[e, ib])

        # ---------------- stage 3: down GEMMs, experts interleaved per tq,
        # each 128-token chunk reduce-scattered as soon as it finalizes ----
        do_g3 = stages >= 3
        split_rs = use_rs and do_g3
        if split_rs:
            y_full = [dram.tile([P, H], F32, name=f"y_full{tt}")
                      for tt in range(NTT)]
            y_rs = [dram.tile([P // ncores, H], F32, name=f"y_rs{tt}")
                    for tt in range(NTT)]
        if do_g3:
            for tq in range(NTT):
                for e in range(n_exp):
                    for nq in range(NHQ):
                        psy = ps.tile([P, 512], F32, tag="y",
                                      name=f"psy{e}_{tq}_{nq}")
                        for ib in range(NIB):
                            nc.tensor.matmul(
                                psy, hhs[e][:, ib, tq * P:(tq + 1) * P],
                                bds[e][:, ib, nq * 512:(nq + 1) * 512],
                                start=(ib == 0), stop=(ib == NIB - 1))
                        nc.vector.scalar_tensor_tensor(
                            out=accs[tq][:, nq * 512:(nq + 1) * 512],
                            in0=psy, scalar=coefsel[e][:, tq:tq + 1],
                            in1=accs[tq][:, nq * 512:(nq + 1) * 512],
                            op0=OP.mult, op1=OP.add)
                if split_rs:
                    nc.sync.dma_start(out=y_full[tq][:, :], in_=accs[tq])
                    nc.gpsimd.collective_compute(
                        "ReduceScatter", OP.add,
                        replica_groups=[list(range(ncores))],
                        ins=[y_full[tq].opt()], outs=[y_rs[tq].opt()])
                    nc.sync.dma_start(out=out_d.ap()[tq], in_=y_rs[tq][:, :])
        if not use_rs:
            for tt in range(NTT):
                nc.sync.dma_start(out=out_d.ap()[tt * P:(tt + 1) * P, :],
                                  in_=accs[tt])


_NC_CACHE = {}


def _get_nc(use_rs=True, stages=4, ncores=NCORES):
    key = (use_rs, stages, ncores)
    if key not in _NC_CACHE:
        _NC_CACHE[key] = build_nc(use_rs, stages, ncores)
    return _NC_CACHE[key]


def _in_maps(inputs, ncores=NCORES):
    import ml_dtypes
    bf16 = ml_dtypes.bfloat16
    epc = E // ncores
    x = np.ascontiguousarray(inputs["hidden_states"], dtype=np.float32)
    gw = np.ascontiguousarray(inputs["gate_weight"], dtype=np.float32)
    cb = np.ascontiguousarray(inputs["correction_bias"], dtype=np.float32)
    wg = np.asarray(inputs["w_gate"], dtype=np.float32).astype(bf16)
    wu = np.asarray(inputs["w_up"], dtype=np.float32).astype(bf16)
    wd = np.asarray(inputs["w_down"], dtype=np.float32).astype(bf16)
    maps = []
    for c in range(ncores):
        sl = slice(c * epc, (c + 1) * epc)
        # [e, i, h] -> [e, ib, hh, kt, ii]
        wgt = np.ascontiguousarray(
            wg[sl].reshape(epc, NIB, P, NKT, P).transpose(0, 1, 4, 3, 2))
        wut = np.ascontiguousarray(
            wu[sl].reshape(epc, NIB, P, NKT, P).transpose(0, 1, 4, 3, 2))
        # [e, h, i] -> [e, ib, ii, h]
        wdt = np.ascontiguousarray(
            wd[sl].reshape(epc, H, NIB, P).transpose(0, 2, 3, 1))
        esel = np.zeros((epc, E), np.float32)
        for el in range(epc):
            esel[el, c * epc + el] = RSF
        maps.append({
            "x": x, "gw": gw, "cb": cb, "esel": esel,
            "wgt": wgt, "wut": wut, "wdt": wdt,
        })
    return maps


def run(inputs, trace=False, use_rs=True, stages=4, ncores=NCORES):
    nc = _get_nc(use_rs, stages, ncores)
    res = run_bass_kernel_spmd(nc, _in_maps(inputs, ncores),
                               core_ids=list(range(ncores)), trace=trace)
    if use_rs:
        # shard[c][tq, r, :] holds tokens tq*128 + c*16 + r
        sh = np.stack([np.asarray(res.results[c]["out_shard"],
                                  dtype=np.float32) for c in range(ncores)])
        out = sh.transpose(1, 0, 2, 3).reshape(T, H)
    else:
        out = np.sum([res.results[c]["out_partial"] for c in range(ncores)],
                     axis=0).astype(np.float32)
    return out, res


def kernel(**inputs) -> np.ndarray:
    out, _ = run(inputs)
    return out
